# revision 3
# baseline (speedup 1.0000x reference)
"""KANLinear forward on 8 TRN2 NeuronCores.

Reference computes
    out = x @ base_w.T + base_b + spline_w @ linspace(0, 1, S)
The spline branch is batch-independent, so it folds into a single bias
vector on the host. The device kernel is a data-parallel matmul: each
core computes a [2048, 1024] batch shard as out.T tiles ([out-feature
partitions, batch free dim]) so the per-feature bias is a per-partition
scalar add fused into the PSUM->SBUF eviction.

Inputs are pre-tiled on the host into the exact SBUF layouts so every
DMA is a contiguous >=2KB-per-partition-line transfer:
  x  -> [NB, 128, KO, 512]   (nb b-tile, ki partition, ko k-subtile, b col)
  w  -> [MO, 128, KO, 128]   (mo o-tile, ki partition, ko k-subtile, m col)
Matmuls run in float32r (TF32-like, 1 row/cycle at N=512) with fp32 PSUM
accumulation.
"""

import numpy as np

import concourse.bass as bass  # noqa: F401  (AP helpers live here)
import concourse.mybir as mybir
import concourse.tile as tile
from concourse import bacc
from concourse.bass_utils import run_bass_kernel_spmd

B, IN, OUT = 16384, 1024, 1024
N_CORES = 8
BS = B // N_CORES  # 2048 batch rows per core
P = 128  # SBUF partitions
KO = IN // P  # 8 k-subtiles of the contraction dim
MO = OUT // P  # 8 out-feature tiles (psum partition dim)
NB_TILE = 512  # matmul free dim = one fp32 PSUM bank
NB = BS // NB_TILE  # 4 batch tiles per core

_CACHE = {}


def _build_nc():
    f32 = mybir.dt.float32
    f32r = mybir.dt.float32r

    nc = bacc.Bacc("TRN2", target_bir_lowering=False)
    x_d = nc.dram_tensor("x_t", [NB, P, KO, NB_TILE], f32r, kind="ExternalInput")
    w_d = nc.dram_tensor("w_t", [MO, P, KO, P], f32r, kind="ExternalInput")
    b_d = nc.dram_tensor("bias_t", [P, MO], f32, kind="ExternalInput")
    o_d = nc.dram_tensor("out_t", [MO, NB, P, NB_TILE], f32, kind="ExternalOutput")

    with tile.TileContext(nc) as tc:
        with (
            tc.tile_pool(name="wp", bufs=1) as wp,
            tc.tile_pool(name="xp", bufs=1) as xp,
            tc.tile_pool(name="cp", bufs=1) as cp,
            tc.tile_pool(name="op", bufs=4) as op,
            tc.tile_pool(name="ps", bufs=4, space="PSUM") as ps,
        ):
            bias_sb = cp.tile([P, MO], f32)
            nc.sync.dma_start(bias_sb[:], b_d[:])

            w_sb = []
            for mo in range(MO):
                t = wp.tile([P, KO, P], f32r, tag=f"w{mo}")
                nc.sync.dma_start(t[:], w_d[mo])
                w_sb.append(t)
            x_sb = []
            for nb in range(NB):
                t = xp.tile([P, KO, NB_TILE], f32r, tag=f"x{nb}")
                nc.sync.dma_start(t[:], x_d[nb])
                x_sb.append(t)

            for nb in range(NB):
                for mo in range(MO):
                    pt = ps.tile([P, NB_TILE], mybir.dt.float32)
                    for k in range(KO):
                        nc.tensor.matmul(
                            pt[:],
                            w_sb[mo][:, k],
                            x_sb[nb][:, k],
                            start=(k == 0),
                            stop=(k == KO - 1),
                        )
                    ot = op.tile([P, NB_TILE], f32)
                    nc.vector.tensor_scalar_add(ot[:], pt[:], bias_sb[:, mo : mo + 1])
                    nc.sync.dma_start(o_d[mo, nb], ot[:])

    nc.finalize()
    return nc


def _get_nc():
    if "nc" not in _CACHE:
        _CACHE["nc"] = _build_nc()
    return _CACHE["nc"]


def _prep_inputs(x, base_w, base_b, spline_w):
    x = np.ascontiguousarray(x, dtype=np.float32)
    base_w = np.ascontiguousarray(base_w, dtype=np.float32)
    base_b = np.ascontiguousarray(base_b, dtype=np.float32)
    spline_w = np.ascontiguousarray(spline_w, dtype=np.float32)

    s_feats = spline_w.shape[1]
    spline_input = np.linspace(0.0, 1.0, s_feats, dtype=np.float32)
    bias = (base_b + spline_w @ spline_input).astype(np.float32)  # [OUT]

    # w_dev[mo, ki, ko, m] = base_w[mo*P + m, ko*P + ki]
    w_dev = np.ascontiguousarray(
        base_w.reshape(MO, P, KO, P).transpose(0, 3, 2, 1)
    )
    # bias_dev[p, mo] = bias[mo*P + p]
    bias_dev = np.ascontiguousarray(bias.reshape(MO, P).T)

    in_maps = []
    for c in range(N_CORES):
        xs = x[c * BS : (c + 1) * BS]  # [BS, IN]
        # x_dev[nb, ki, ko, col] = xs[nb*NB_TILE + col, ko*P + ki]
        x_dev = np.ascontiguousarray(
            xs.reshape(NB, NB_TILE, KO, P).transpose(0, 3, 2, 1)
        )
        in_maps.append({"x_t": x_dev, "w_t": w_dev, "bias_t": bias_dev})
    return in_maps


def _run(inputs, trace=False, tmpdir=None):
    nc = _get_nc()
    in_maps = _prep_inputs(**inputs)
    res = run_bass_kernel_spmd(
        nc, in_maps, core_ids=list(range(N_CORES)), trace=trace, tmpdir=tmpdir
    )
    outs = []
    for c in range(N_CORES):
        arr = np.asarray(res.results[c]["out_t"])  # [MO, NB, P, NB_TILE]
        # out_core[nb*NB_TILE + col, mo*P + p] = arr[mo, nb, p, col]
        outs.append(
            arr.transpose(1, 3, 0, 2).reshape(BS, OUT)
        )
    full = np.ascontiguousarray(np.concatenate(outs, axis=0), dtype=np.float32)
    return full, res


def kernel(**inputs) -> np.ndarray:
    out, _ = _run(inputs, trace=False)
    return out


# revision 5
# speedup vs baseline: 1.0586x; 1.0586x over previous
"""KANLinear forward on 8 TRN2 NeuronCores.

Reference computes
    out = x @ base_w.T + base_b + spline_w @ linspace(0, 1, S)
The spline branch is batch-independent, so it folds into a single bias
vector on the host. The device kernel is a data-parallel matmul: each
core computes a [2048, 1024] batch shard as out.T tiles ([out-feature
partitions, batch free dim]) so the per-feature bias is a per-partition
scalar add fused into the PSUM->SBUF eviction.

Inputs are pre-tiled on the host into the exact SBUF layouts so every
DMA is a contiguous >=2KB-per-partition-line transfer:
  x  -> [NB, 128, KO, 512]   (nb b-tile, ki partition, ko k-subtile, b col)
  w  -> [MO, 128, KO, 128]   (mo o-tile, ki partition, ko k-subtile, m col)
Matmuls run in float32r (TF32-like, 1 row/cycle at N=512) with fp32 PSUM
accumulation.
"""

import numpy as np

import concourse.bass as bass  # noqa: F401  (AP helpers live here)
import concourse.mybir as mybir
import concourse.tile as tile
from concourse import bacc
from concourse.bass_utils import run_bass_kernel_spmd

B, IN, OUT = 16384, 1024, 1024
N_CORES = 8
BS = B // N_CORES  # 2048 batch rows per core
P = 128  # SBUF partitions
KO = IN // P  # 8 k-subtiles of the contraction dim
MO = OUT // P  # 8 out-feature tiles (psum partition dim)
NB_TILE = 512  # matmul free dim = one fp32 PSUM bank
NB = BS // NB_TILE  # 4 batch tiles per core

_CACHE = {}


def _build_nc():
    f32 = mybir.dt.float32
    f32r = mybir.dt.float32r

    nc = bacc.Bacc("TRN2", target_bir_lowering=False)
    x_d = nc.dram_tensor("x_t", [NB, P, KO, NB_TILE], f32r, kind="ExternalInput")
    w_d = nc.dram_tensor("w_t", [MO, P, KO, P], f32r, kind="ExternalInput")
    b_d = nc.dram_tensor("bias_t", [P, MO], f32, kind="ExternalInput")
    o_d = nc.dram_tensor("out_t", [MO, NB, P, NB_TILE], f32, kind="ExternalOutput")

    with tile.TileContext(nc) as tc:
        with (
            tc.tile_pool(name="wp", bufs=1) as wp,
            tc.tile_pool(name="xp", bufs=1) as xp,
            tc.tile_pool(name="cp", bufs=1) as cp,
            tc.tile_pool(name="op", bufs=4) as op,
            tc.tile_pool(name="ps", bufs=4, space="PSUM") as ps,
        ):
            bias_sb = cp.tile([P, MO], f32)
            nc.sync.dma_start(bias_sb[:], b_d[:])

            KH = KO // 2  # x chunks split in k-halves for earlier PE start
            w_sb = [None] * MO
            x_sb = [[None, None] for _ in range(NB)]

            def load_w(mo):
                t = wp.tile([P, KO, P], f32r, tag=f"w{mo}")
                nc.sync.dma_start(t[:], w_d[mo])
                w_sb[mo] = t

            def load_x(nb, h):
                t = xp.tile([P, KH, NB_TILE], f32r, tag=f"x{nb}_{h}")
                nc.sync.dma_start(t[:], x_d[nb, :, h * KH : (h + 1) * KH])
                x_sb[nb][h] = t

            # DMA issue order follows PE consumption order so the PE can
            # start as soon as the first w/x chunks land.
            load_w(0)
            load_x(0, 0)
            load_x(0, 1)
            for mo in range(1, MO):
                load_w(mo)
            for nb in range(1, NB):
                load_x(nb, 0)
                load_x(nb, 1)

            for nb in range(NB):
                for mo in range(MO):
                    pt = ps.tile([P, NB_TILE], mybir.dt.float32)
                    for k in range(KO):
                        nc.tensor.matmul(
                            pt[:],
                            w_sb[mo][:, k],
                            x_sb[nb][k // KH][:, k % KH],
                            start=(k == 0),
                            stop=(k == KO - 1),
                        )
                    ot = op.tile([P, NB_TILE], f32)
                    nc.vector.tensor_scalar_add(ot[:], pt[:], bias_sb[:, mo : mo + 1])
                    nc.sync.dma_start(o_d[mo, nb], ot[:])

    nc.finalize()
    return nc


def _get_nc():
    if "nc" not in _CACHE:
        _CACHE["nc"] = _build_nc()
    return _CACHE["nc"]


def _prep_inputs(x, base_w, base_b, spline_w):
    x = np.ascontiguousarray(x, dtype=np.float32)
    base_w = np.ascontiguousarray(base_w, dtype=np.float32)
    base_b = np.ascontiguousarray(base_b, dtype=np.float32)
    spline_w = np.ascontiguousarray(spline_w, dtype=np.float32)

    s_feats = spline_w.shape[1]
    spline_input = np.linspace(0.0, 1.0, s_feats, dtype=np.float32)
    bias = (base_b + spline_w @ spline_input).astype(np.float32)  # [OUT]

    # w_dev[mo, ki, ko, m] = base_w[mo*P + m, ko*P + ki]
    w_dev = np.ascontiguousarray(
        base_w.reshape(MO, P, KO, P).transpose(0, 3, 2, 1)
    )
    # bias_dev[p, mo] = bias[mo*P + p]
    bias_dev = np.ascontiguousarray(bias.reshape(MO, P).T)

    in_maps = []
    for c in range(N_CORES):
        xs = x[c * BS : (c + 1) * BS]  # [BS, IN]
        # x_dev[nb, ki, ko, col] = xs[nb*NB_TILE + col, ko*P + ki]
        x_dev = np.ascontiguousarray(
            xs.reshape(NB, NB_TILE, KO, P).transpose(0, 3, 2, 1)
        )
        in_maps.append({"x_t": x_dev, "w_t": w_dev, "bias_t": bias_dev})
    return in_maps


def _run(inputs, trace=False, tmpdir=None):
    nc = _get_nc()
    in_maps = _prep_inputs(**inputs)
    res = run_bass_kernel_spmd(
        nc, in_maps, core_ids=list(range(N_CORES)), trace=trace, tmpdir=tmpdir
    )
    outs = []
    for c in range(N_CORES):
        arr = np.asarray(res.results[c]["out_t"])  # [MO, NB, P, NB_TILE]
        # out_core[nb*NB_TILE + col, mo*P + p] = arr[mo, nb, p, col]
        outs.append(
            arr.transpose(1, 3, 0, 2).reshape(BS, OUT)
        )
    full = np.ascontiguousarray(np.concatenate(outs, axis=0), dtype=np.float32)
    return full, res


def kernel(**inputs) -> np.ndarray:
    out, _ = _run(inputs, trace=False)
    return out
